# revision 46
# baseline (speedup 1.0000x reference)
"""Single-head causal attention on 8 TRN2 NeuronCores.

out[b,t,:] = softmax_causal((x Wq^T)(x Wk^T)^T / sqrt(C)) @ (x Wv^T)

Sharding: core = (batch b=core//2, parity p=core%2). Each core owns the
interleaved q-512-blocks g in {p, p+2, p+4, p+6} of its batch. One uniform
SPMD program: per q-slot i the main (strictly-below-diagonal) phase runs a
fixed EMAIN[i] = [4,12,20,28] key-chunk extents; parity-0 cores get 4
zero-padded key chunks prepended host-side, killed by a per-partition -BIG
bias fused into the exp activation (pads -> exp -> 0), so they contribute
exactly zero to both numerator and denominator (ones column in v storage).

Pipeline structure (single in-order PE stream, ~everything overlapped):
 - inputs DMA'd in per-512-block chunks ordered by first use (first two
   blocks split in half), so the first projection starts right after the
   fixed preamble and all later loads hide under compute.
 - projections M-packed: (q|k) and (k|v) as single M=128 matmuls.
 - v^T -> v natural via PE transposes sharing the projection psum ring.
 - attention emitted software-pipelined in units (main chunk-pairs: two
   score matmuls into one [128,1024] psum, ONE exp; diagonal singles
   column-shrunk to skip fully-masked queries), PV lagging 2 units, with
   projection work for later q-blocks interleaved as fillers into the
   ACT(exp)-bound stretches; triangular window masked on the Pool engine.
Scores are produced transposed (S^T[s,tq]) so the softmax denominator
falls out of the PV matmul's extra ones column; no probability transposes.
All matmul operands bf16; accumulation fp32; no max-subtraction (scores
are O(1) for this distribution; exp is safe).
"""

import math
import os
import sys

for _p in ("/opt/trn_rl_repo",):
    if _p not in sys.path:
        sys.path.insert(0, _p)

import numpy as np
import ml_dtypes

BF16 = ml_dtypes.bfloat16

B, T, C, H = 4, 4096, 1024, 64
NCORES = 8
SCALE = C ** -0.5
NEGBIG = -30000.0

QB = 512                    # q block width (columns of q^T per block)
NQB = 4                     # q blocks per core (4 * 512 = 2048 rows)
EMAIN = (4, 12, 20, 28)     # uniform main-phase extents (128-key chunks)
MAINC = 28                  # main kv chunks per core (28*128 = 3584 cols)
DIAGC = 16                  # diag kv chunks per core (owns its 2048 q rows)
VN = MAINC + DIAGC          # 44 chunks in v-natural storage
NQ = NQB * QB               # 2048
NM = MAINC * 128            # 3584

_CACHE = {}


def _build_program():
    import concourse.bass as bass
    import concourse.mybir as mybir
    import concourse.tile as tile
    from concourse import bacc
    from concourse.masks import make_identity

    f32 = mybir.dt.float32
    bf16 = mybir.dt.bfloat16

    nc = bacc.Bacc("TRN2", target_bir_lowering=False, debug=False)
    # x pre-interleaved host-side, block-major: block g cols [g*4096,
    # (g+1)*4096) hold its 8 C-chunks of 512 q-cols each.
    xq_d = nc.dram_tensor("xq", [128, NQB * 4096], bf16,
                          kind="ExternalInput")
    xkv_d = nc.dram_tensor("xkv", [128, MAINC // 4 * 4096], bf16,
                           kind="ExternalInput")
    # w pre-interleaved host-side into the SBUF layout [128, 8*192]:
    # chunk c cols [c*192,(c+1)*192) = [Wq^T | Wk^T | Wv^T] rows 128c..
    wt_d = nc.dram_tensor("wt", [128, 8 * 192], bf16, kind="ExternalInput")
    bias_d = nc.dram_tensor("bias", [128, 4], f32, kind="ExternalInput")
    out_d = nc.dram_tensor("out", [NQ, H], f32, kind="ExternalOutput")

    with tile.TileContext(nc) as tc:
        with tc.tile_pool(name="persist", bufs=1) as P, \
             tc.tile_pool(name="psum", bufs=1, space="PSUM") as PS, \
             tc.tile_pool(name="work", bufs=1) as W:
            # ---- persistent SBUF -----------------------------------------
            xq_sb = P.tile([128, 8 * NQ], bf16)
            xkv_sb = P.tile([128, 8 * NM], bf16)
            w_sb = P.tile([128, 8 * 192], bf16)
            qT_sb = P.tile([64, NQ], bf16)        # q^T
            kq_sb = P.tile([64, NQ], bf16)        # diag k^T
            kv_sb = P.tile([128, NM], bf16)       # rows 0-63 k^T, 64-127 v^T
            vq_sb = P.tile([64, NQ], bf16)        # diag v^T staging
            vn_sb = P.tile([128, VN * 80], bf16)  # v nat (64) + ones col @64
            tri_sb = P.tile([128, 128], bf16)     # causal window (keep t>=s)
            idf_sb = P.tile([128, 128], f32)      # f32 identity (out transp)
            idb_sb = P.tile([128, 128], bf16)     # bf16 identity (v transp)
            bias_sb = P.tile([128, 4], f32)       # exp bias (-BIG on pads)

            scr_sb = P.tile([64, 512], bf16)      # p-state warmup operand

            # ---- constants -----------------------------------------------
            nc.gpsimd.memset(scr_sb[:, :], 0.0)
            make_identity(nc, idf_sb[:, :])
            make_identity(nc, idb_sb[:, :])
            nc.gpsimd.memset(
                vn_sb.rearrange("p (j s) -> p j s", j=VN)[:, :, 64:65], 1.0)
            nc.gpsimd.memset(tri_sb[:, :], 1.0)
            nc.gpsimd.affine_select(
                out=tri_sb[:, :], in_=tri_sb[:, :],
                compare_op=mybir.AluOpType.is_ge, fill=0.0,
                base=0, pattern=[[1, 128]], channel_multiplier=-1)

            # ---- input DMAs: per-512-block chunks, ordered by first use.
            # Full blocks move as one contiguous [128, 4096] transfer; the
            # two split half-blocks use a strided per-chunk AP.
            def dma_xq(g, lo=0, hi=QB, eng=None):
                eng = eng or nc.sync
                if lo == 0 and hi == QB:
                    eng.dma_start(
                        out=xq_sb[:, g * 4096:(g + 1) * 4096],
                        in_=xq_d[:, g * 4096:(g + 1) * 4096])
                    return
                o3 = xq_sb[:, g * 4096:(g + 1) * 4096].rearrange(
                    "p (c n) -> p c n", c=8)[:, :, lo:hi]
                i3 = xq_d[:, g * 4096:(g + 1) * 4096].rearrange(
                    "p (c n) -> p c n", c=8)[:, :, lo:hi]
                nc.sync.dma_start(out=o3, in_=i3)

            def dma_kv(b, lo=0, hi=512, eng=None):
                eng = eng or nc.sync
                if lo == 0 and hi == 512:
                    eng.dma_start(
                        out=xkv_sb[:, b * 4096:(b + 1) * 4096],
                        in_=xkv_d[:, b * 4096:(b + 1) * 4096])
                    return
                o3 = xkv_sb[:, b * 4096:(b + 1) * 4096].rearrange(
                    "p (c n) -> p c n", c=8)[:, :, lo:hi]
                i3 = xkv_d[:, b * 4096:(b + 1) * 4096].rearrange(
                    "p (c n) -> p c n", c=8)[:, :, lo:hi]
                nc.sync.dma_start(out=o3, in_=i3)

            # w/bias go out on the ACT engine's DMA queue so their issue
            # and transfer overlap the x loads on the sync queue.
            nc.scalar.dma_start(out=w_sb, in_=wt_d[:, :])
            nc.scalar.dma_start(out=bias_sb, in_=bias_d[:, :])
            dma_kv(0, 0, 256)
            dma_kv(0, 256, 512)
            dma_xq(0, 0, 256)
            dma_xq(0, 256, 512)
            dma_xq(1)
            dma_kv(1)
            dma_kv(2)
            dma_xq(2)
            dma_kv(3)
            dma_kv(4)
            dma_xq(3)
            dma_kv(5)
            dma_kv(6)

            # v^T chunk [64,128] -> v natural [128,64] via PE transpose into
            # a pt-ring bank (bf16 view of the f32 tile), then copy into vn.
            def mk_vtr(j, src_ap, hi):
                def f():
                    tp = PS.tile([128, 65], f32, tag="fin", bufs=1,
                                 name="vtr")
                    tb = tp[:, 0:32].bitcast(mybir.dt.bfloat16)  # [128,64]
                    ident = idb_sb[64:128, 64:128] if hi \
                        else idb_sb[0:64, 0:64]
                    nc.tensor.transpose(tb, src_ap, ident)
                    nc.vector.tensor_copy(
                        vn_sb[:, j * 80:j * 80 + 64], tb)
                return f

            # ---- projection emitters (filler items: (weight, closure)) ---
            # w_sb layout per chunk c: [q 0:64 | k 64:128 | v 128:192]
            def proj_xq_items(g, lo=0, hi=QB):
                items = []
                pt = {}
                span = hi - lo

                def mk_qk(c):
                    def f():
                        if c == 0:
                            pt["qk"] = PS.tile([128, 512], f32, tag="pt",
                                               bufs=2, name="ptqk")
                        nc.tensor.matmul(
                            pt["qk"][:, 0:span],
                            w_sb[:, c * 192:c * 192 + 128],
                            xq_sb[:, g * 4096 + c * 512 + lo:
                                  g * 4096 + c * 512 + hi],
                            start=(c == 0), stop=(c == 7))
                    return f

                def mk_v(c):
                    def f():
                        if c == 0:
                            pt["v"] = PS.tile([128, 512], f32, tag="pt",
                                              bufs=2, name="ptv")
                        nc.tensor.matmul(
                            pt["v"][0:64, 0:span],
                            w_sb[:, c * 192 + 128:c * 192 + 192],
                            xq_sb[:, g * 4096 + c * 512 + lo:
                                  g * 4096 + c * 512 + hi],
                            start=(c == 0), stop=(c == 7))
                    return f

                def cp_qk():
                    nc.vector.tensor_copy(
                        qT_sb[0:64, g * QB + lo:g * QB + hi],
                        pt["qk"][0:64, 0:span])
                    nc.vector.tensor_copy(
                        kq_sb[0:64, g * QB + lo:g * QB + hi],
                        pt["qk"][64:128, 0:span])

                def cp_v():
                    nc.vector.tensor_copy(
                        vq_sb[0:64, g * QB + lo:g * QB + hi],
                        pt["v"][0:64, 0:span])

                for c in range(8):
                    items.append((1, mk_qk(c)))
                items.append((0, cp_qk))
                for c in range(8):
                    items.append((1, mk_v(c)))
                items.append((0, cp_v))
                for d in range(lo // 128, hi // 128):
                    j = 4 * g + d
                    items.append((1, mk_vtr(
                        MAINC + j, vq_sb[0:64, j * 128:(j + 1) * 128],
                        False)))
                return items

            def proj_kv_items(b, lo=0, hi=512):
                items = []
                pt = {}
                span = hi - lo

                def mk(c):
                    def f():
                        if c == 0:
                            pt["kv"] = PS.tile([128, 512], f32, tag="pt",
                                               bufs=2, name="ptkv")
                        nc.tensor.matmul(
                            pt["kv"][:, 0:span],
                            w_sb[:, c * 192 + 64:c * 192 + 192],
                            xkv_sb[:, b * 4096 + c * 512 + lo:
                                   b * 4096 + c * 512 + hi],
                            start=(c == 0), stop=(c == 7))
                    return f

                def cp():
                    nc.vector.tensor_copy(
                        kv_sb[:, b * 512 + lo:b * 512 + hi],
                        pt["kv"][:, 0:span])

                for c in range(8):
                    items.append((1, mk(c)))
                items.append((0, cp))
                for cc in range(lo // 128, hi // 128):
                    j = 4 * b + cc
                    items.append((1, mk_vtr(
                        j, kv_sb[64:128, j * 128:(j + 1) * 128], True)))
                return items

            # ---- finalize emitter (shares the pt psum ring) --------------
            def fin_items(g, op):
                items = []
                st = {}

                def mk_t4(t4):
                    def f():
                        if t4 == 0:
                            st["ob"] = W.tile([65, 512], f32, tag="ob",
                                              bufs=2, name="ob")
                        nc.vector.tensor_copy(
                            st["ob"][:, t4 * 128:(t4 + 1) * 128],
                            op[0:65, t4 * 128:(t4 + 1) * 128])
                        if g == 3 and t4 % 2 == 1:
                            # attention is over: the sc ring is idle, use it
                            # to double up the tail transposes
                            tpw = PS.tile([128, 1024], f32, tag="sc",
                                          bufs=2, name="otrs")
                            tp = tpw[:, 0:65]
                        else:
                            tp = PS.tile([128, 65], f32, tag="fin", bufs=1,
                                         name="otr")
                        nc.tensor.transpose(
                            tp, st["ob"][:, t4 * 128:(t4 + 1) * 128],
                            idf_sb[0:65, 0:65])
                        rc = W.tile([128, 1], f32, tag="rc", bufs=2,
                                    name="rc")
                        nc.vector.reciprocal(rc, tp[:, 64:65])
                        rs = W.tile([128, 64], f32, tag="rs", bufs=2,
                                    name="rs")
                        nc.scalar.mul(rs, tp[:, 0:64], rc)
                        r0 = g * QB + t4 * 128
                        nc.sync.dma_start(out=out_d[r0:r0 + 128, :], in_=rs)
                    return f

                for t4 in range(4):
                    items.append((1, mk_t4(t4)))
                return items

            # ---- attention -----------------------------------------------
            # Units per qb g: EMAIN[g]/2 main PAIRS (two 128-key chunks, two
            # score matmuls into one [128,1024] psum, ONE exp) then 4 diag
            # singles. Diag d covers local keys [128d,128d+128); only
            # queries t >= 128d see it: score/exp/PV shrink to W=512-128d.
            def emit_unit(g, unit, op, first, last):
                kind, t = unit
                sc = PS.tile([128, 1024], f32, tag="sc", bufs=2, name="sc")
                pb = W.tile([128, 1024], bf16, tag="pb", bufs=4, name="pb")
                qAP = qT_sb[0:64, g * QB:(g + 1) * QB]
                if kind == "pair":
                    for u in range(2):
                        nc.tensor.matmul(
                            sc[:, u * 512:(u + 1) * 512],
                            kv_sb[0:64, (t + u) * 128:(t + u + 1) * 128],
                            qAP, start=True, stop=True)
                    bias = bias_sb[:, t:t + 1] if t < 4 else 0.0
                    nc.scalar.activation(
                        pb, sc, mybir.ActivationFunctionType.Exp,
                        scale=SCALE, bias=bias)

                    def pv():
                        for u in range(2):
                            j = t + u
                            nc.tensor.matmul(
                                op[0:65, :], vn_sb[:, j * 80:j * 80 + 65],
                                pb[:, u * 512:(u + 1) * 512],
                                start=(first and u == 0), stop=False,
                                skip_group_check=True)
                    return pv
                d = t
                w = 512 - 128 * d
                j = 4 * g + d
                nc.tensor.matmul(
                    sc[:, 0:w], kq_sb[0:64, j * 128:(j + 1) * 128],
                    qT_sb[0:64, g * QB + 128 * d:(g + 1) * QB],
                    start=True, stop=True)
                nc.scalar.activation(
                    pb[:, 0:w], sc[:, 0:w],
                    mybir.ActivationFunctionType.Exp, scale=SCALE)
                nc.vector.tensor_mul(pb[:, 0:128], pb[:, 0:128], tri_sb)

                def pv():
                    jj = MAINC + 4 * g + d
                    nc.tensor.matmul(
                        op[0:65, 128 * d:512],
                        vn_sb[:, jj * 80:jj * 80 + 65],
                        pb[:, 0:w], start=False, stop=last,
                        skip_group_check=True)
                return pv

            # ---- top-level schedule --------------------------------------
            for lo, hi in ((0, 256), (256, 512)):
                for _, f in proj_kv_items(0, lo, hi):
                    f()
            for lo, hi in ((0, 256), (256, 512)):
                for _, f in proj_xq_items(0, lo, hi):
                    f()

            def weave(*lists):
                # lists: item-lists ending in vtr runs; push each run into
                # the following list's head, one vtr per two items.
                out = []
                carry = []
                for L in lists:
                    body, i = list(L), 0
                    woven = []
                    for it in body:
                        woven.append(it)
                        if carry and len(woven) % 3 == 0:
                            woven.append(carry.pop(0))
                    woven.extend(carry)
                    carry = []
                    # peel this list's trailing vtr run (last 4 weight-1
                    # items after the final copy) to weave into the next
                    tail = []
                    while woven and len(tail) < 4 and woven[-1][0] == 1:
                        tail.insert(0, woven.pop())
                    carry = tail
                    out.extend(woven)
                out.extend(carry)
                return out

            fillers = []        # (weight, closure) queue, consumed in order

            def pump(target):
                while fillers and pump.done < target:
                    wgt, f = fillers.pop(0)
                    f()
                    pump.done += wgt
                while fillers and fillers[0][0] == 0:
                    _, f = fillers.pop(0)
                    f()
            pump.done = 0

            qb_fill = {
                0: lambda: weave(proj_xq_items(1), proj_kv_items(1),
                                 proj_kv_items(2)),
                1: lambda: weave(proj_xq_items(2), proj_kv_items(3),
                                 proj_kv_items(4)),
                2: lambda: weave(proj_xq_items(3)),
                3: lambda: weave(proj_kv_items(5), proj_kv_items(6)),
            }
            # for qb3 the kv5/kv6 fillers must land before chunk-slots 20/24
            # (units 10/12): pace them over the first 9 units.
            pace_units = {0: 6, 1: 10, 2: 14, 3: 9}

            for g in range(NQB):
                op = PS.tile([128, 512], f32, tag="op", bufs=1, name="op")
                units = [("pair", t) for t in range(0, EMAIN[g], 2)] \
                    + [("diag", d) for d in range(4)]
                n = len(units)
                base = pump.done
                fillers.extend(qb_fill[g]())
                wsum = sum(w for w, _ in fillers)
                pend = []   # pv closures awaiting emission (lag 2 units)
                npace = pace_units[g]
                for i, unit in enumerate(units):
                    pump(base + math.ceil(wsum * min(1.0, (i + 1) / npace)))
                    if len(pend) >= 3:
                        pend.pop(0)()
                    pend.append(emit_unit(g, unit, op, i == 0, i == n - 1))
                for pv in pend:
                    pv()
                fillers.extend(fin_items(g, op))

            while fillers:
                _, f = fillers.pop(0)
                f()
    nc.compile()
    return nc


def _get_program():
    if "nc" not in _CACHE:
        _CACHE["nc"] = _build_program()
    return _CACHE["nc"]


def _host_prep(x, Wk, Wq, Wv):
    """Build the 8 per-core input maps."""
    wt = np.concatenate([Wq.T, Wk.T, Wv.T], axis=1).astype(BF16)  # [C, 192]
    wt_il = np.concatenate(
        [wt[c * 128:(c + 1) * 128, :] for c in range(8)], axis=1)
    xT = [np.ascontiguousarray(x[b].T).astype(BF16) for b in range(B)]
    bias_pad = np.full((128, 4), NEGBIG, dtype=np.float32)
    bias_real = np.zeros((128, 4), dtype=np.float32)

    def interleave(cols):
        # [C, n512*512] -> block-major [128, n512*4096]: per 512-block the
        # 8 C-chunks of [128, 512] laid out side by side.
        n512 = cols.shape[1] // 512
        out = np.empty((128, n512 * 4096), dtype=BF16)
        for g in range(n512):
            blk = cols[:, g * 512:(g + 1) * 512].reshape(8, 128, 512)
            out[:, g * 4096:(g + 1) * 4096] = \
                blk.transpose(1, 0, 2).reshape(128, 4096)
        return out

    in_maps = []
    for core in range(NCORES):
        b, p = core // 2, core % 2
        gs = [2 * i + p for i in range(NQB)]
        xq = np.concatenate(
            [xT[b][:, 512 * g:512 * g + 512] for g in gs], axis=1)
        if p == 0:
            xkv = np.concatenate(
                [np.zeros((C, 512), dtype=BF16), xT[b][:, 0:3072]], axis=1)
            bias = bias_pad
        else:
            xkv = np.ascontiguousarray(xT[b][:, 0:3584])
            bias = bias_real
        in_maps.append({
            "xq": interleave(xq),
            "xkv": interleave(xkv),
            "wt": np.ascontiguousarray(wt_il),
            "bias": bias,
        })
    return in_maps


def _gather(results):
    out = np.zeros((B, T, H), dtype=np.float32)
    for core in range(NCORES):
        b, p = core // 2, core % 2
        shard = np.asarray(results[core]["out"], dtype=np.float32)
        for i in range(NQB):
            g = 2 * i + p
            out[b, 512 * g:512 * g + 512, :] = shard[512 * i:512 * i + 512, :]
    return out


def run(x, Wk, Wq, Wv, trace=False):
    from concourse.bass_utils import run_bass_kernel_spmd

    nc = _get_program()
    in_maps = _host_prep(x, Wk, Wq, Wv)
    res = run_bass_kernel_spmd(
        nc, in_maps, list(range(NCORES)), trace=trace)
    return _gather(res.results), res


def kernel(x, Wk, Wq, Wv):
    out, _ = run(np.asarray(x, dtype=np.float32),
                 np.asarray(Wk, dtype=np.float32),
                 np.asarray(Wq, dtype=np.float32),
                 np.asarray(Wv, dtype=np.float32))
    return out
